# revision 11
# baseline (speedup 1.0000x reference)
import numpy as np
import concourse.bass as bass
import concourse.bacc as bacc
import concourse.mybir as mybir
import concourse.tile as tile
from concourse.bass_utils import run_bass_kernel_spmd

NCORES = 8
N = 15546          # nodes
F = 3000           # input features
FP = 3072          # padded (24 chunks of 128); row 3000 = ones for bias
KCH = FP // 128    # 24
H = 64
O = 4
R = 3
NPC = 1944         # nodes per core (core 7 holds 1938 real)
PADN = 2048        # padded per-core node count (16 blocks of 128)
NBLK = PADN // 128  # 16
TBL = NCORES * PADN  # 16384 table rows
DEC = 100000       # decode edges
DPC = DEC // NCORES  # 12500
DPAD = 12544       # padded decode edges per core (98 * 128)
DCH = DPAD // 128  # 98

F32 = mybir.dt.float32
F16 = mybir.dt.float16
I16 = mybir.dt.int16

TRACE = False
BENCH = 0
LAST_RESULT = None
LAST_TIMES = None
GS = 8             # gather chunks per dma_gather call (1024 idxs)
NQ = 4             # swdge queues


def _build(CH, ch_counts, dbg=0):
    nc = bacc.Bacc("TRN2", target_bir_lowering=False, debug=False,
                   num_devices=NCORES, num_swdge_queues=NQ)
    xt = nc.dram_tensor("xt", [128, NBLK, KCH, 128], F16,
                        kind="ExternalInput").ap()
    wc = nc.dram_tensor("wc", [128, KCH, 128], F16,
                        kind="ExternalInput").ap()
    oh1_in = nc.dram_tensor("oh1_in", [128, CH, 128], F16,
                            kind="ExternalInput").ap()
    oh2_in = nc.dram_tensor("oh2_in", [128, CH, 128], F16,
                            kind="ExternalInput").ap()
    gidx_in = nc.dram_tensor("gidx_in", [128, CH * 8], I16,
                             kind="ExternalInput").ap()
    d0_in = nc.dram_tensor("d0_in", [128, DPAD // 16], I16,
                           kind="ExternalInput").ap()
    d1_in = nc.dram_tensor("d1_in", [128, DPAD // 16], I16,
                           kind="ExternalInput").ap()
    r2e_in = nc.dram_tensor("r2e_in", [H + 1, O], F16,
                            kind="ExternalInput").ap()
    b2_in = nc.dram_tensor("b2_in", [H, O], F16,
                           kind="ExternalInput").ap()
    wt_in = nc.dram_tensor("wt_in", [1, O * O], F32,
                           kind="ExternalInput").ap()
    sig_out = nc.dram_tensor("sig", [128, DCH], F32,
                             kind="ExternalOutput").ap()
    if dbg:
        dbg_out = nc.dram_tensor("dbg", [128, NBLK, 128], F32,
                                 kind="ExternalOutput").ap()

    eq = mybir.AluOpType.is_equal
    mul = mybir.AluOpType.mult
    add = mybir.AluOpType.add
    mx = mybir.AluOpType.max
    AX = mybir.AxisListType.X
    AF = mybir.ActivationFunctionType

    with tile.TileContext(nc) as tc:
        with tc.tile_pool(name="dram", bufs=1, space="DRAM") as dram, \
             tc.tile_pool(name="sb", bufs=1) as sb, \
             tc.tile_pool(name="ps", bufs=1, space="PSUM") as ps:
            # ---- constants ----
            ii32 = sb.tile([128, 128], mybir.dt.int32, tag="ii32")
            nc.gpsimd.iota(ii32[:], pattern=[[1, 128]], base=0,
                           channel_multiplier=0)
            iota_f = sb.tile([128, 128], F32, tag="iota_f")
            nc.vector.tensor_copy(iota_f[:], ii32[:])
            pi32 = sb.tile([128, 1], mybir.dt.int32, tag="pi32")
            nc.gpsimd.iota(pi32[:], pattern=[[1, 1]], base=0,
                           channel_multiplier=1)
            pif = sb.tile([128, 1], F32, tag="pif")
            nc.vector.tensor_copy(pif[:], pi32[:])
            ident = sb.tile([128, 128], F32, tag="ident")
            nc.vector.tensor_scalar(ident[:], iota_f[:], pif[:], None, eq)
            ident16 = sb.tile([128, 128], F16, tag="ident16")
            nc.vector.tensor_copy(ident16[:], ident[:])
            ones1 = sb.tile([1, 128], F32, tag="ones1")
            nc.vector.memset(ones1[:], 1.0)

            # ---- small inputs ----
            r2e = sb.tile([H + 1, O], F16, tag="r2e")
            nc.sync.dma_start(r2e[:], r2e_in[:])
            b2s = sb.tile([H, O], F16, tag="b2s")
            nc.sync.dma_start(b2s[:], b2_in[:])
            wtf = sb.tile([1, O * O], F32, tag="wtf")
            nc.sync.dma_start(wtf[:], wt_in[:])
            gix = sb.tile([128, CH * 8], I16, tag="gix")
            nc.gpsimd.dma_start(gix[:], gidx_in[:])
            d0x = sb.tile([128, DPAD // 16], I16, tag="d0x")
            nc.gpsimd.dma_start(d0x[:], d0_in[:])
            d1x = sb.tile([128, DPAD // 16], I16, tag="d1x")
            nc.gpsimd.dma_start(d1x[:], d1_in[:])
            wcs = sb.tile([128, KCH, 128], F16, tag="wcs")
            nc.scalar.dma_start(wcs[:], wc[:])

            # W broadcast: wb[p, i*4+j] = W[i, j] for all p
            wbp = ps.tile([128, 128], F32, tag="pmm", bufs=2)
            nc.tensor.matmul(wbp[:, 0:O * O], ones1[:], wtf[:], start=True,
                             stop=True)
            wb = sb.tile([128, O * O], F32, tag="wb")
            nc.scalar.activation(wb[:], wbp[:, 0:O * O], AF.Copy)

            # ---- persistent state ----
            uv_sb = sb.tile([128, NBLK, 128], F16, tag="uv_sb")
            h_sb = sb.tile([128, NBLK, 128], F16, tag="h_sb")
            hT = sb.tile([H + 1, NBLK, 128], F16, tag="hT")
            nc.vector.memset(hT[H:H + 1, :, :], 1.0)
            zpre = sb.tile([128, NBLK, O], F32, tag="zpre")
            zq_sb = sb.tile([128, NBLK, H], F32, tag="zq_sb")
            nc.vector.memset(zq_sb[:], 0.0)

            # ---- shared DRAM ----
            u_loc = dram.tile([128, NBLK, 128], F16, tag="u_loc")
            h_loc = dram.tile([128, NBLK, 128], F16, tag="h_loc")
            zq_loc = dram.tile([128, NBLK, H], F32, tag="zq_loc")
            u_sh = dram.tile([TBL, 128], F16, tag="u_sh",
                             addr_space="Shared")
            h_sh = dram.tile([TBL, 128], F16, tag="h_sh",
                             addr_space="Shared")
            zq_sh = dram.tile([TBL, H], F32, tag="zq_sh",
                              addr_space="Shared")

            # ---- projection: uv[p, b, :] = [x@B1 | x@root1+bias1] ----
            engs = (nc.sync, nc.scalar, nc.gpsimd)
            for b in range(NBLK):
                xtb = sb.tile([128, KCH, 128], F16, tag="xtb", bufs=6)
                engs[b % 3].dma_start(xtb[:], xt[:, b])
                pp = ps.tile([128, 128], F32, tag="pmm", bufs=2)
                for k in range(KCH):
                    nc.tensor.matmul(pp[:], xtb[:, k, :], wcs[:, k, :],
                                     start=(k == 0), stop=(k == KCH - 1))
                nc.scalar.activation(uv_sb[:, b, :], pp[:], AF.Copy)
                nc.sync.dma_start(u_loc[:, b, :], uv_sb[:, b, :])
            oh1 = sb.tile([128, CH, 128], F16, tag="oh1")
            nc.sync.dma_start(oh1[:], oh1_in[:])
            oh2 = sb.tile([128, CH, 128], F16, tag="oh2")
            nc.scalar.dma_start(oh2[:], oh2_in[:])
            if dbg == 1:
                dbf = sb.tile([128, NBLK, 128], F32, tag="dbf")
                nc.vector.tensor_copy(dbf[:], uv_sb[:])
                nc.sync.dma_start(dbg_out[:], dbf[:])

            nc.gpsimd.collective_compute(
                "AllGather", mybir.AluOpType.bypass,
                replica_groups=[list(range(NCORES))],
                ins=[u_loc.opt()], outs=[u_sh.opt()])

            # ---- layer 1 (slab-pipelined gather + per-block agg) ----
            chunk0 = np.zeros(NBLK, dtype=int)
            chunk0[1:] = np.cumsum(ch_counts)[:-1]
            nslab = -(-CH // GS)

            def _full_gather(tag, table):
                t = sb.tile([128, CH, 128], F16, tag=tag, bufs=2)
                for i, c0 in enumerate(range(0, CH, GS)):
                    c1 = min(CH, c0 + GS)
                    nc.gpsimd.dma_gather(
                        out_ap=t[:, c0:c1, :], in_ap=table[:],
                        idxs_ap=gix[:, c0 * 8:c1 * 8],
                        num_idxs=(c1 - c0) * 128,
                        num_idxs_reg=(c1 - c0) * 128, elem_size=128,
                        single_packet=True, queue_num=i % NQ)
                return t

            def _l1_block(b, g):
                nch = ch_counts[b]
                hb = ps.tile([128, 128], F32, tag="pmm", bufs=2)
                for j in range(nch):
                    cv = chunk0[b] + j
                    nc.tensor.matmul(hb[:, 0:H], oh1[:, cv, :],
                                     g[:, cv, 0:H],
                                     start=(j == 0), stop=(j == nch - 1))
                nc.vector.tensor_tensor(h_sb[:, b, 0:H], hb[:, 0:H],
                                        uv_sb[:, b, H:128], op=add)
                nc.vector.tensor_scalar_max(h_sb[:, b, 0:H],
                                            h_sb[:, b, 0:H], 0.0)
                (nc.sync if b % 2 else nc.scalar).dma_start(
                    h_loc[:, b, :], h_sb[:, b, :])
                ph = ps.tile([H, 128], F16, tag="ph", bufs=1)
                nc.tensor.transpose(ph[:], h_sb[:, b, 0:H], ident16[:])
                nc.scalar.activation(hT[0:H, b, :], ph[:], AF.Copy)

            def _l2_block(b, g):
                nch = ch_counts[b]
                pa2 = ps.tile([H, 128], F32, tag="pa2", bufs=2)
                for j in range(nch):
                    cv = chunk0[b] + j
                    nc.tensor.matmul(pa2[:], g[:, cv, 0:H],
                                     oh2[:, cv, :],
                                     start=(j == 0), stop=(j == nch - 1))
                at2 = sb.tile([H, 128], F16, tag="at2", bufs=2)
                nc.scalar.activation(at2[:], pa2[:], AF.Copy)
                pd = ps.tile([O, 128], F32, tag="pd", bufs=2)
                nc.tensor.matmul(pd[:], r2e[:], hT[:, b, :],
                                 start=True, stop=False)
                nc.tensor.matmul(pd[:], b2s[:], at2[:],
                                 start=False, stop=True)
                pds = sb.tile([O, 128], F32, tag="pds", bufs=2)
                nc.vector.tensor_copy(pds[:], pd[:])
                ztp = ps.tile([128, O], F32, tag="ztp", bufs=1)
                nc.tensor.transpose(ztp[:], pds[:], ident[0:O, 0:O])
                nc.scalar.activation(zpre[:, b, :], ztp[:], AF.Copy)

            def _layer(table, blockfn, tag):
                g = _full_gather(tag, table)
                for bi in range(NBLK):
                    blockfn(bi, g)

            _layer(u_sh, _l1_block, "gbuf")
            if dbg == 2:
                dbf = sb.tile([128, NBLK, 128], F32, tag="dbf")
                nc.vector.tensor_copy(dbf[:], h_sb[:])
                nc.sync.dma_start(dbg_out[:], dbf[:])

            nc.gpsimd.collective_compute(
                "AllGather", mybir.AluOpType.bypass,
                replica_groups=[list(range(NCORES))],
                ins=[h_loc.opt()], outs=[h_sh.opt()])

            _layer(h_sh, _l2_block, "gbuf")

            # ---- batched softmax over O=4 (per node) ----
            nm = sb.tile([128, NBLK, 1], F32, tag="nm")
            nc.vector.tensor_reduce(nm[:], zpre[:], axis=AX, op=mx,
                                    negate=True)
            zs = sb.tile([128, NBLK, O], F32, tag="zs")
            nc.vector.tensor_tensor(zs[:], zpre[:],
                                    nm[:].to_broadcast([128, NBLK, O]),
                                    op=add)
            ez = sb.tile([128, NBLK, O], F32, tag="ez")
            nc.scalar.activation(ez[:], zs[:], AF.Exp)
            ssum = sb.tile([128, NBLK, 1], F32, tag="ssum")
            nc.vector.tensor_reduce(ssum[:], ez[:], axis=AX, op=add)
            rc = sb.tile([128, NBLK, 1], F32, tag="rc")
            nc.vector.reciprocal(rc[:], ssum[:])
            nc.vector.tensor_tensor(zq_sb[:, :, 0:O], ez[:],
                                    rc[:].to_broadcast([128, NBLK, O]),
                                    op=mul)
            # ---- q = W @ z per node, via per-scalar DVE ----
            qk = sb.tile([128, NBLK, 1], F32, tag="qk")
            for i in range(O):
                nc.vector.tensor_scalar(zq_sb[:, :, O + i:O + i + 1],
                                        zq_sb[:, :, 0:1],
                                        wb[:, 4 * i:4 * i + 1], None, mul)
                for j in range(1, O):
                    nc.vector.tensor_scalar(qk[:], zq_sb[:, :, j:j + 1],
                                            wb[:, 4 * i + j:4 * i + j + 1],
                                            None, mul)
                    nc.vector.tensor_tensor(zq_sb[:, :, O + i:O + i + 1],
                                            zq_sb[:, :, O + i:O + i + 1],
                                            qk[:], op=add)
            if dbg == 3:
                dbf = sb.tile([128, NBLK, 128], F32, tag="dbf")
                nc.vector.memset(dbf[:], 0.0)
                nc.vector.tensor_copy(dbf[:, :, 0:H], zq_sb[:])
                nc.sync.dma_start(dbg_out[:], dbf[:])
            nc.sync.dma_start(zq_loc[:], zq_sb[:])

            nc.gpsimd.collective_compute(
                "AllGather", mybir.AluOpType.bypass,
                replica_groups=[list(range(NCORES))],
                ins=[zq_loc.opt()], outs=[zq_sh.opt()])

            # ---- decode: sigmoid(dot(z[d0], q[d1])), slab-pipelined ----
            qi = 0
            for c0 in range(0, DCH, GS):
                c1 = min(c0 + GS, DCH)
                w = c1 - c0
                sl = []
                for tg, idxs in (("gd0", d0x), ("gd1", d1x)):
                    t = sb.tile([128, GS, H], F32, tag=tg, bufs=5)
                    nc.gpsimd.dma_gather(
                        out_ap=t[:, 0:w, :], in_ap=zq_sh[:],
                        idxs_ap=idxs[:, c0 * 8:c1 * 8],
                        num_idxs=w * 128,
                        num_idxs_reg=w * 128, elem_size=H,
                        single_packet=True, queue_num=qi % NQ)
                    qi += 1
                    sl.append(t)
                pr = sb.tile([128, GS, O], F32, tag="pr", bufs=3)
                nc.vector.tensor_tensor(pr[:, 0:w, :], sl[0][:, 0:w, 0:O],
                                        sl[1][:, 0:w, O:2 * O], op=mul)
                lg = sb.tile([128, GS], F32, tag="lg", bufs=3)
                nc.vector.tensor_reduce(lg[:, 0:w], pr[:, 0:w, :],
                                        axis=AX, op=add)
                sg = sb.tile([128, GS], F32, tag="sg", bufs=3)
                nc.scalar.activation(sg[:, 0:w], lg[:, 0:w], AF.Sigmoid)
                nc.sync.dma_start(sig_out[:, c0:c1], sg[:, 0:w])
    nc.finalize()
    return nc


def _wrap_idx(flat):
    # device reads idx for flat slot i at sbuf[i % 16, i // 16],
    # replicated across the 8 gpsimd cores (partition groups of 16)
    n = flat.shape[0]
    w = flat.reshape(n // 16, 16).T.astype(np.int16)
    return np.tile(w, (8, 1))


def _prep(inputs):
    x = np.asarray(inputs["x"], dtype=np.float32)
    comp1 = np.asarray(inputs["comp1"], dtype=np.float32)[:, 0]
    bases1 = np.asarray(inputs["bases1"], dtype=np.float32)[0]
    root1 = np.asarray(inputs["root1"], dtype=np.float32)
    bias1 = np.asarray(inputs["bias1"], dtype=np.float32)
    comp2 = np.asarray(inputs["comp2"], dtype=np.float32)[:, 0]
    bases2 = np.asarray(inputs["bases2"], dtype=np.float32)[0]
    root2 = np.asarray(inputs["root2"], dtype=np.float32)
    bias2 = np.asarray(inputs["bias2"], dtype=np.float32)
    bil_w = np.asarray(inputs["bil_w"], dtype=np.float32)[0]
    ei = np.asarray(inputs["edge_index"], dtype=np.int64)
    et = np.asarray(inputs["edge_type"], dtype=np.int64)
    pos = np.asarray(inputs["pos_edge_index"], dtype=np.int64)
    neg = np.asarray(inputs["neg_edge_index"], dtype=np.int64)

    src, tgt = ei[0], ei[1]

    # ---- per-edge folded weights: comp[et] / max(cnt[tgt, et], 1) ----
    seg = tgt * R + et
    cnt = np.bincount(seg, minlength=N * R).astype(np.float32)
    denom = np.maximum(cnt, 1.0)[seg]
    w1 = comp1[et] / denom
    w2 = comp2[et] / denom

    # ---- node position remap into [128, 16] per-core table layout ----
    nn = np.arange(N, dtype=np.int64)
    cc = nn // NPC
    li = nn - cc * NPC
    remap = cc * PADN + (li % 128) * NBLK + (li // 128)  # [N] < 16384

    # ---- partition edges by (target core, target block) ----
    core = tgt // NPC
    tli = tgt - core * NPC
    blk = tli // 128
    lt = (tli % 128).astype(np.int64)
    key = core * NBLK + blk
    order = np.argsort(key, kind="stable")
    counts2d = np.bincount(key, minlength=NCORES * NBLK).reshape(
        NCORES, NBLK)
    ch_counts = np.maximum(1, -(-counts2d.max(axis=0) // 128)).astype(int)
    CH = int(ch_counts.sum())
    chunk0 = np.zeros(NBLK, dtype=int)
    chunk0[1:] = np.cumsum(ch_counts)[:-1]
    starts = np.zeros(NCORES * NBLK + 1, dtype=int)
    starts[1:] = np.cumsum(counts2d.reshape(-1))

    L = CH * 128
    gsrc_pos = remap[src]
    in_maps = []
    dec = np.concatenate([pos, neg], axis=1)
    wcat = np.zeros((FP, 128), dtype=np.float32)
    wcat[:F, 0:H] = bases1
    wcat[:F, H:128] = root1
    wcat[F, H:128] = bias1
    wc_dev = np.ascontiguousarray(
        wcat.reshape(KCH, 128, 128).transpose(1, 0, 2)).astype(np.float16)
    r2e_dev = np.zeros((H + 1, O), dtype=np.float16)
    r2e_dev[:H] = root2.astype(np.float16)
    r2e_dev[H] = bias2.astype(np.float16)
    b2_dev = bases2.astype(np.float16)
    wt_dev = np.ascontiguousarray(bil_w.reshape(1, O * O))

    for c in range(NCORES):
        ltf = np.zeros(L, np.int64)
        w1f = np.zeros(L, np.float32)
        w2f = np.zeros(L, np.float32)
        gsf = np.zeros(L, np.int64)
        for b in range(NBLK):
            kidx = c * NBLK + b
            sl = order[starts[kidx]:starts[kidx + 1]]
            off = chunk0[b] * 128
            ltf[off:off + len(sl)] = lt[sl]
            w1f[off:off + len(sl)] = w1[sl]
            w2f[off:off + len(sl)] = w2[sl]
            gsf[off:off + len(sl)] = gsrc_pos[sl]

        # baked one-hots: oh[slot, t] = w[slot] * (lt[slot] == t)
        oh1 = np.zeros((L, 128), np.float16)
        oh1[np.arange(L), ltf] = w1f.astype(np.float16)
        oh2 = np.zeros((L, 128), np.float16)
        oh2[np.arange(L), ltf] = w2f.astype(np.float16)
        oh1_dev = np.ascontiguousarray(
            oh1.reshape(CH, 128, 128).transpose(1, 0, 2))
        oh2_dev = np.ascontiguousarray(
            oh2.reshape(CH, 128, 128).transpose(1, 0, 2))

        nreal = min(NPC, N - c * NPC)
        xp = np.zeros((FP, PADN), dtype=np.float32)
        xp[:F, :nreal] = x[c * NPC:c * NPC + nreal].T
        xp[F, :nreal] = 1.0
        xt_dev = np.ascontiguousarray(
            xp.reshape(KCH, 128, NBLK, 128).transpose(1, 2, 0, 3)
        ).astype(np.float16)

        d0f = np.zeros(DPAD, np.int64)
        d1f = np.zeros(DPAD, np.int64)
        d0f[:DPC] = remap[dec[0, c * DPC:(c + 1) * DPC]]
        d1f[:DPC] = remap[dec[1, c * DPC:(c + 1) * DPC]]

        in_maps.append({
            "xt": xt_dev,
            "wc": wc_dev,
            "oh1_in": oh1_dev,
            "oh2_in": oh2_dev,
            "gidx_in": _wrap_idx(gsf),
            "d0_in": _wrap_idx(d0f),
            "d1_in": _wrap_idx(d1f),
            "r2e_in": r2e_dev,
            "b2_in": b2_dev,
            "wt_in": wt_dev,
        })
    return in_maps, CH, ch_counts


def _bench(nc, in_maps, iters=20):
    import time as _time
    import jax
    from jax.sharding import Mesh, PartitionSpec, NamedSharding
    from jax.experimental.shard_map import shard_map
    from concourse import bass2jax as b2j

    b2j.install_neuronx_cc_hook()
    pname = nc.partition_id_tensor.name if nc.partition_id_tensor else None
    in_names, out_names, out_avals, zero_outs = [], [], [], []
    for alloc in nc.m.functions[0].allocations:
        if not isinstance(alloc, mybir.MemoryLocationSet):
            continue
        name = alloc.memorylocations[0].name
        if alloc.kind == "ExternalInput":
            if name != pname:
                in_names.append(name)
        elif alloc.kind == "ExternalOutput":
            shape = tuple(alloc.tensor_shape)
            dtype = mybir.dt.np(alloc.dtype)
            out_names.append(name)
            out_avals.append(jax.core.ShapedArray(shape, dtype))
            zero_outs.append(np.zeros(shape, dtype))
    n_params = len(in_names)
    n_outs = len(out_avals)
    in_names.extend(out_names)
    if pname is not None:
        in_names.append(pname)

    def _body(*args):
        operands = list(args)
        if pname is not None:
            operands.append(b2j.partition_id_tensor())
        return tuple(b2j._bass_exec_p.bind(
            *operands, out_avals=tuple(out_avals), in_names=tuple(in_names),
            out_names=tuple(out_names), lowering_input_output_aliases=(),
            sim_require_finite=True, sim_require_nnan=True, nc=nc))

    devices = jax.devices()[:NCORES]
    mesh = Mesh(np.asarray(devices), ("core",))
    specs = (PartitionSpec("core"),)
    fn = jax.jit(shard_map(_body, mesh=mesh,
                           in_specs=specs * (n_params + n_outs),
                           out_specs=specs * n_outs, check_rep=False),
                 keep_unused=True)
    concat_in = [np.concatenate([np.asarray(in_maps[c][nm])
                                 for c in range(NCORES)], axis=0)
                 for nm in in_names[:n_params]]
    sh = NamedSharding(mesh, PartitionSpec("core"))
    dev_in = [jax.device_put(a, sh) for a in concat_in]
    dev_zero = [jax.device_put(
        np.zeros((NCORES * z.shape[0], *z.shape[1:]), z.dtype), sh)
        for z in zero_outs]
    jax.block_until_ready(dev_in)
    jax.block_until_ready(dev_zero)
    # Under the axon tunnel a blocking round trip costs ~40-85 ms of pure
    # network latency regardless of kernel content, so per-iteration
    # blocking wall time measures the tunnel, not the hardware. Amortize:
    # dispatch a deep pipeline of executions, divide total wall time.
    t0 = _time.perf_counter()
    outs = fn(*dev_in, *dev_zero)
    jax.block_until_ready(outs)
    times = [_time.perf_counter() - t0]
    npipe = max(1024, iters)
    for _ in range(5):
        t0 = _time.perf_counter()
        allouts = [fn(*dev_in, *dev_zero) for _ in range(npipe)]
        jax.block_until_ready(allouts)
        times.append((_time.perf_counter() - t0) / npipe)
        del allouts
    del outs
    return times


def kernel(**inputs):
    in_maps, CH, ch_counts = _prep(inputs)
    nc = _build(CH, ch_counts)
    res = run_bass_kernel_spmd(nc, in_maps, core_ids=list(range(NCORES)),
                               trace=TRACE)
    globals()["LAST_RESULT"] = res
    if BENCH:
        times = _bench(nc, in_maps, iters=BENCH)
        globals()["LAST_TIMES"] = times

    out = np.empty(DEC, dtype=np.float32)
    for c in range(NCORES):
        arr = res.results[c]["sig"]  # [128, DCH]; slot s=ch*128+p -> edge s
        out[c * DPC:(c + 1) * DPC] = arr.T.reshape(-1)[:DPC]
    return out


# revision 12
# speedup vs baseline: 1.4086x; 1.4086x over previous
import numpy as np
import concourse.bass as bass
import concourse.bacc as bacc
import concourse.mybir as mybir
import concourse.tile as tile
from concourse.bass_utils import run_bass_kernel_spmd

NCORES = 8
N = 15546          # nodes
F = 3000           # input features
FP = 3072          # padded (24 chunks of 128); row 3000 = ones for bias
KCH = FP // 128    # 24
H = 64
O = 4
R = 3
NPC = 1944         # nodes per core (core 7 holds 1938 real)
PADN = 2048        # padded per-core node count (16 blocks of 128)
NBLK = PADN // 128  # 16
TBL = NCORES * PADN  # 16384 table rows
DEC = 100000       # decode edges
DPC = DEC // NCORES  # 12500
DPAD = 12544       # padded decode edges per core (98 * 128)
DCH = DPAD // 128  # 98

F32 = mybir.dt.float32
F16 = mybir.dt.float16
I16 = mybir.dt.int16

TRACE = False
BENCH = 0
LAST_RESULT = None
LAST_TIMES = None
GS = 8             # gather chunks per dma_gather call (1024 idxs)
NQ = 4             # swdge queues


def _a64(x):
    return (x + 63) & ~63


def _offsets(CH):
    XT0 = 0
    WC0 = XT0 + NBLK * KCH * 128          # 49152
    LT0 = WC0 + KCH * 128                 # +3072
    W10 = LT0 + _a64(CH)
    W20 = W10 + _a64(CH)
    GI0 = W20 + _a64(CH)
    D00 = GI0 + _a64(CH * 8)
    D10 = D00 + _a64(DPAD // 16)
    R20 = D10 + _a64(DPAD // 16)
    B20 = R20 + 64
    WT0 = B20 + 64
    TOTC = WT0 + 64
    return XT0, WC0, LT0, W10, W20, GI0, D00, D10, R20, B20, WT0, TOTC


def _build(CH, ch_counts, dbg=0):
    XT0, WC0, LT0, W10, W20, GI0, D00, D10, R20, B20, WT0, TOTC = \
        _offsets(CH)
    nc = bacc.Bacc("TRN2", target_bir_lowering=False, debug=False,
                   num_devices=NCORES, num_swdge_queues=NQ)
    blob = nc.dram_tensor("blob", [128, TOTC], F16,
                          kind="ExternalInput").ap()
    sig_out = nc.dram_tensor("sig", [128, DCH], F32,
                             kind="ExternalOutput").ap()
    if dbg:
        dbg_out = nc.dram_tensor("dbg", [128, NBLK, 128], F32,
                                 kind="ExternalOutput").ap()

    eq = mybir.AluOpType.is_equal
    mul = mybir.AluOpType.mult
    add = mybir.AluOpType.add
    mx = mybir.AluOpType.max
    AX = mybir.AxisListType.X
    AF = mybir.ActivationFunctionType

    with tile.TileContext(nc) as tc:
        with tc.tile_pool(name="dram", bufs=1, space="DRAM") as dram, \
             tc.tile_pool(name="sb", bufs=1) as sb, \
             tc.tile_pool(name="ps", bufs=1, space="PSUM") as ps:
            # ---- constants ----
            ii32 = sb.tile([128, 128], mybir.dt.int32, tag="ii32")
            nc.gpsimd.iota(ii32[:], pattern=[[1, 128]], base=0,
                           channel_multiplier=0)
            iota_f = sb.tile([128, 128], F32, tag="iota_f")
            nc.vector.tensor_copy(iota_f[:], ii32[:])
            pi32 = sb.tile([128, 1], mybir.dt.int32, tag="pi32")
            nc.gpsimd.iota(pi32[:], pattern=[[1, 1]], base=0,
                           channel_multiplier=1)
            pif = sb.tile([128, 1], F32, tag="pif")
            nc.vector.tensor_copy(pif[:], pi32[:])
            ident = sb.tile([128, 128], F32, tag="ident")
            nc.vector.tensor_scalar(ident[:], iota_f[:], pif[:], None, eq)
            ident16 = sb.tile([128, 128], F16, tag="ident16")
            nc.vector.tensor_copy(ident16[:], ident[:])
            ones1 = sb.tile([1, 128], F32, tag="ones1")
            nc.vector.memset(ones1[:], 1.0)

            # ---- small inputs (sliced out of the blob) ----
            r2e = sb.tile([128, O], F16, tag="r2e")
            nc.sync.dma_start(r2e[:], blob[:, R20:R20 + O])
            b2s = sb.tile([128, O], F16, tag="b2s")
            nc.sync.dma_start(b2s[:], blob[:, B20:B20 + O])
            wtf = sb.tile([1, O * O], F32, tag="wtf")
            nc.sync.dma_start(wtf[:], blob[0:1, WT0:WT0 + 32].bitcast(F32))
            gix = sb.tile([128, CH * 8], I16, tag="gix")
            nc.gpsimd.dma_start(gix[:],
                                blob[:, GI0:GI0 + CH * 8].bitcast(I16))
            d0x = sb.tile([128, DPAD // 16], I16, tag="d0x")
            nc.gpsimd.dma_start(d0x[:],
                                blob[:, D00:D00 + DPAD // 16].bitcast(I16))
            d1x = sb.tile([128, DPAD // 16], I16, tag="d1x")
            nc.gpsimd.dma_start(d1x[:],
                                blob[:, D10:D10 + DPAD // 16].bitcast(I16))
            lt16 = sb.tile([128, 3 * CH], F16, tag="lt16")
            nc.sync.dma_start(lt16[:, 0:CH], blob[:, LT0:LT0 + CH])
            nc.sync.dma_start(lt16[:, CH:2 * CH], blob[:, W10:W10 + CH])
            nc.sync.dma_start(lt16[:, 2 * CH:3 * CH],
                              blob[:, W20:W20 + CH])
            ltw = sb.tile([128, 3 * CH], F32, tag="ltw")
            nc.vector.tensor_copy(ltw[:], lt16[:])
            wcs = sb.tile([128, KCH * 128], F16, tag="wcs")
            nc.scalar.dma_start(wcs[:], blob[:, WC0:WC0 + KCH * 128])

            # W broadcast: wb[p, i*4+j] = W[i, j] for all p
            wbp = ps.tile([128, 128], F32, tag="pmm", bufs=2)
            nc.tensor.matmul(wbp[:, 0:O * O], ones1[:], wtf[:], start=True,
                             stop=True)
            wb = sb.tile([128, O * O], F32, tag="wb")
            nc.scalar.activation(wb[:], wbp[:, 0:O * O], AF.Copy)

            # ---- on-device one-hot build (DVE, hidden under proj DMA) ----
            oh1 = sb.tile([128, CH, 128], F16, tag="oh1")
            oh2 = sb.tile([128, CH, 128], F16, tag="oh2")
            for cv in range(CH):
                nc.vector.tensor_scalar(oh1[:, cv, :], iota_f[:],
                                        ltw[:, cv:cv + 1],
                                        ltw[:, CH + cv:CH + cv + 1],
                                        eq, mul)
            for cv in range(CH):
                nc.vector.tensor_scalar(oh2[:, cv, :], iota_f[:],
                                        ltw[:, cv:cv + 1],
                                        ltw[:, 2 * CH + cv:2 * CH + cv + 1],
                                        eq, mul)

            # ---- persistent state ----
            uv_sb = sb.tile([128, NBLK, 128], F16, tag="uv_sb")
            h_sb = sb.tile([128, NBLK, 128], F16, tag="h_sb")
            hT = sb.tile([H + 1, NBLK, 128], F16, tag="hT")
            nc.vector.memset(hT[H:H + 1, :, :], 1.0)
            zpre = sb.tile([128, NBLK, O], F32, tag="zpre")
            zq_sb = sb.tile([128, NBLK, H], F32, tag="zq_sb")
            nc.vector.memset(zq_sb[:], 0.0)

            # ---- shared DRAM ----
            u_loc = dram.tile([128, NBLK, 128], F16, tag="u_loc")
            h_loc = dram.tile([128, NBLK, 128], F16, tag="h_loc")
            zq_loc = dram.tile([128, NBLK, H], F32, tag="zq_loc")
            u_sh = dram.tile([TBL, 128], F16, tag="u_sh",
                             addr_space="Shared")
            h_sh = dram.tile([TBL, 128], F16, tag="h_sh",
                             addr_space="Shared")
            zq_sh = dram.tile([TBL, H], F32, tag="zq_sh",
                              addr_space="Shared")

            # ---- projection: uv[p, b, :] = [x@B1 | x@root1+bias1] ----
            engs = (nc.sync, nc.scalar, nc.gpsimd)
            for b in range(NBLK):
                xtb = sb.tile([128, KCH * 128], F16, tag="xtb", bufs=6)
                engs[b % 3].dma_start(
                    xtb[:], blob[:, XT0 + b * KCH * 128:
                                 XT0 + (b + 1) * KCH * 128])
                pp = ps.tile([128, 128], F32, tag="pmm", bufs=2)
                for k in range(KCH):
                    nc.tensor.matmul(pp[:], xtb[:, k * 128:(k + 1) * 128],
                                     wcs[:, k * 128:(k + 1) * 128],
                                     start=(k == 0), stop=(k == KCH - 1))
                nc.scalar.activation(uv_sb[:, b, :], pp[:], AF.Copy)
                nc.sync.dma_start(u_loc[:, b, :], uv_sb[:, b, :])
            if dbg == 1:
                dbf = sb.tile([128, NBLK, 128], F32, tag="dbf")
                nc.vector.tensor_copy(dbf[:], uv_sb[:])
                nc.sync.dma_start(dbg_out[:], dbf[:])

            nc.gpsimd.collective_compute(
                "AllGather", mybir.AluOpType.bypass,
                replica_groups=[list(range(NCORES))],
                ins=[u_loc.opt()], outs=[u_sh.opt()])

            # ---- layers ----
            chunk0 = np.zeros(NBLK, dtype=int)
            chunk0[1:] = np.cumsum(ch_counts)[:-1]

            def _full_gather(tag, table):
                t = sb.tile([128, CH, 128], F16, tag=tag, bufs=2)
                for i, c0 in enumerate(range(0, CH, GS)):
                    c1 = min(CH, c0 + GS)
                    nc.gpsimd.dma_gather(
                        out_ap=t[:, c0:c1, :], in_ap=table[:],
                        idxs_ap=gix[:, c0 * 8:c1 * 8],
                        num_idxs=(c1 - c0) * 128,
                        num_idxs_reg=(c1 - c0) * 128, elem_size=128,
                        single_packet=True, queue_num=i % NQ)
                return t

            def _l1_block(b, g):
                nch = ch_counts[b]
                hb = ps.tile([128, 128], F32, tag="pmm", bufs=2)
                for j in range(nch):
                    cv = chunk0[b] + j
                    nc.tensor.matmul(hb[:, 0:H], oh1[:, cv, :],
                                     g[:, cv, 0:H],
                                     start=(j == 0), stop=(j == nch - 1))
                nc.vector.tensor_tensor(h_sb[:, b, 0:H], hb[:, 0:H],
                                        uv_sb[:, b, H:128], op=add)
                nc.vector.tensor_scalar_max(h_sb[:, b, 0:H],
                                            h_sb[:, b, 0:H], 0.0)
                (nc.sync if b % 2 else nc.scalar).dma_start(
                    h_loc[:, b, :], h_sb[:, b, :])
                ph = ps.tile([H, 128], F16, tag="ph", bufs=2)
                nc.tensor.transpose(ph[:], h_sb[:, b, 0:H], ident16[:])
                nc.vector.tensor_copy(hT[0:H, b, :], ph[:])

            def _l2_block(b, g):
                nch = ch_counts[b]
                pa2 = ps.tile([H, 128], F32, tag="pa2", bufs=1)
                for j in range(nch):
                    cv = chunk0[b] + j
                    nc.tensor.matmul(pa2[:], g[:, cv, 0:H],
                                     oh2[:, cv, :],
                                     start=(j == 0), stop=(j == nch - 1))
                at2 = sb.tile([H, 128], F16, tag="at2", bufs=2)
                nc.scalar.activation(at2[:], pa2[:], AF.Copy)
                pd = ps.tile([O, 128], F32, tag="pd", bufs=1)
                nc.tensor.matmul(pd[:], r2e[0:H + 1, :], hT[:, b, :],
                                 start=True, stop=False)
                nc.tensor.matmul(pd[:], b2s[0:H, :], at2[:],
                                 start=False, stop=True)
                pds = sb.tile([O, 128], F32, tag="pds", bufs=2)
                nc.vector.tensor_copy(pds[:], pd[:])
                ztp = ps.tile([128, O], F32, tag="ztp", bufs=2)
                nc.tensor.transpose(ztp[:], pds[:], ident[0:O, 0:O])
                nc.scalar.activation(zpre[:, b, :], ztp[:], AF.Copy)

            g1 = _full_gather("gbuf", u_sh)
            for b in range(NBLK):
                _l1_block(b, g1)
            if dbg == 2:
                dbf = sb.tile([128, NBLK, 128], F32, tag="dbf")
                nc.vector.tensor_copy(dbf[:], h_sb[:])
                nc.sync.dma_start(dbg_out[:], dbf[:])

            nc.gpsimd.collective_compute(
                "AllGather", mybir.AluOpType.bypass,
                replica_groups=[list(range(NCORES))],
                ins=[h_loc.opt()], outs=[h_sh.opt()])

            g2 = _full_gather("gbuf", h_sh)
            for b in range(NBLK):
                _l2_block(b, g2)

            # ---- batched softmax over O=4 (per node) ----
            nm = sb.tile([128, NBLK, 1], F32, tag="nm")
            nc.vector.tensor_reduce(nm[:], zpre[:], axis=AX, op=mx,
                                    negate=True)
            zs = sb.tile([128, NBLK, O], F32, tag="zs")
            nc.vector.tensor_tensor(zs[:], zpre[:],
                                    nm[:].to_broadcast([128, NBLK, O]),
                                    op=add)
            ez = sb.tile([128, NBLK, O], F32, tag="ez")
            nc.scalar.activation(ez[:], zs[:], AF.Exp)
            ssum = sb.tile([128, NBLK, 1], F32, tag="ssum")
            nc.vector.tensor_reduce(ssum[:], ez[:], axis=AX, op=add)
            rc = sb.tile([128, NBLK, 1], F32, tag="rc")
            nc.vector.reciprocal(rc[:], ssum[:])
            nc.vector.tensor_tensor(zq_sb[:, :, 0:O], ez[:],
                                    rc[:].to_broadcast([128, NBLK, O]),
                                    op=mul)
            # ---- q = W @ z per node, via per-scalar DVE ----
            qk = sb.tile([128, NBLK, 1], F32, tag="qk")
            for i in range(O):
                nc.vector.tensor_scalar(zq_sb[:, :, O + i:O + i + 1],
                                        zq_sb[:, :, 0:1],
                                        wb[:, 4 * i:4 * i + 1], None, mul)
                for j in range(1, O):
                    nc.vector.tensor_scalar(qk[:], zq_sb[:, :, j:j + 1],
                                            wb[:, 4 * i + j:4 * i + j + 1],
                                            None, mul)
                    nc.vector.tensor_tensor(zq_sb[:, :, O + i:O + i + 1],
                                            zq_sb[:, :, O + i:O + i + 1],
                                            qk[:], op=add)
            if dbg == 3:
                dbf = sb.tile([128, NBLK, 128], F32, tag="dbf")
                nc.vector.memset(dbf[:], 0.0)
                nc.vector.tensor_copy(dbf[:, :, 0:H], zq_sb[:])
                nc.sync.dma_start(dbg_out[:], dbf[:])
            nc.sync.dma_start(zq_loc[:], zq_sb[:])

            nc.gpsimd.collective_compute(
                "AllGather", mybir.AluOpType.bypass,
                replica_groups=[list(range(NCORES))],
                ins=[zq_loc.opt()], outs=[zq_sh.opt()])

            # ---- decode: sigmoid(dot(z[d0], q[d1])), slab-pipelined ----
            qi = 0
            for c0 in range(0, DCH, GS):
                c1 = min(c0 + GS, DCH)
                w = c1 - c0
                sl = []
                for tg, idxs in (("gd0", d0x), ("gd1", d1x)):
                    t = sb.tile([128, GS, H], F32, tag=tg, bufs=5)
                    nc.gpsimd.dma_gather(
                        out_ap=t[:, 0:w, :], in_ap=zq_sh[:],
                        idxs_ap=idxs[:, c0 * 8:c1 * 8],
                        num_idxs=w * 128,
                        num_idxs_reg=w * 128, elem_size=H,
                        single_packet=True, queue_num=qi % NQ)
                    qi += 1
                    sl.append(t)
                pr = sb.tile([128, GS, O], F32, tag="pr", bufs=3)
                nc.vector.tensor_tensor(pr[:, 0:w, :], sl[0][:, 0:w, 0:O],
                                        sl[1][:, 0:w, O:2 * O], op=mul)
                lg = sb.tile([128, GS], F32, tag="lg", bufs=3)
                nc.vector.tensor_reduce(lg[:, 0:w], pr[:, 0:w, :],
                                        axis=AX, op=add)
                sg = sb.tile([128, GS], F32, tag="sg", bufs=3)
                nc.scalar.activation(sg[:, 0:w], lg[:, 0:w], AF.Sigmoid)
                nc.sync.dma_start(sig_out[:, c0:c1], sg[:, 0:w])
    nc.finalize()
    return nc


def _wrap_idx(flat):
    # device reads idx for flat slot i at sbuf[i % 16, i // 16],
    # replicated across the 8 gpsimd cores (partition groups of 16)
    n = flat.shape[0]
    w = flat.reshape(n // 16, 16).T.astype(np.int16)
    return np.tile(w, (8, 1))


def _prep(inputs):
    x = np.asarray(inputs["x"], dtype=np.float32)
    comp1 = np.asarray(inputs["comp1"], dtype=np.float32)[:, 0]
    bases1 = np.asarray(inputs["bases1"], dtype=np.float32)[0]
    root1 = np.asarray(inputs["root1"], dtype=np.float32)
    bias1 = np.asarray(inputs["bias1"], dtype=np.float32)
    comp2 = np.asarray(inputs["comp2"], dtype=np.float32)[:, 0]
    bases2 = np.asarray(inputs["bases2"], dtype=np.float32)[0]
    root2 = np.asarray(inputs["root2"], dtype=np.float32)
    bias2 = np.asarray(inputs["bias2"], dtype=np.float32)
    bil_w = np.asarray(inputs["bil_w"], dtype=np.float32)[0]
    ei = np.asarray(inputs["edge_index"], dtype=np.int64)
    et = np.asarray(inputs["edge_type"], dtype=np.int64)
    pos = np.asarray(inputs["pos_edge_index"], dtype=np.int64)
    neg = np.asarray(inputs["neg_edge_index"], dtype=np.int64)

    src, tgt = ei[0], ei[1]

    # ---- per-edge folded weights: comp[et] / max(cnt[tgt, et], 1) ----
    seg = tgt * R + et
    cnt = np.bincount(seg, minlength=N * R).astype(np.float32)
    denom = np.maximum(cnt, 1.0)[seg]
    w1 = comp1[et] / denom
    w2 = comp2[et] / denom

    # ---- node position remap into [128, 16] per-core table layout ----
    nn = np.arange(N, dtype=np.int64)
    cc = nn // NPC
    li = nn - cc * NPC
    remap = cc * PADN + (li % 128) * NBLK + (li // 128)  # [N] < 16384

    # ---- partition edges by (target core, target block) ----
    core = tgt // NPC
    tli = tgt - core * NPC
    blk = tli // 128
    lt = (tli % 128).astype(np.int64)
    key = core * NBLK + blk
    order = np.argsort(key, kind="stable")
    counts2d = np.bincount(key, minlength=NCORES * NBLK).reshape(
        NCORES, NBLK)
    ch_counts = np.maximum(1, -(-counts2d.max(axis=0) // 128)).astype(int)
    CH = int(ch_counts.sum())
    chunk0 = np.zeros(NBLK, dtype=int)
    chunk0[1:] = np.cumsum(ch_counts)[:-1]
    starts = np.zeros(NCORES * NBLK + 1, dtype=int)
    starts[1:] = np.cumsum(counts2d.reshape(-1))

    XT0, WC0, LT0, W10, W20, GI0, D00, D10, R20, B20, WT0, TOTC = \
        _offsets(CH)

    L = CH * 128
    gsrc_pos = remap[src]
    in_maps = []
    dec = np.concatenate([pos, neg], axis=1)
    wcat = np.zeros((FP, 128), dtype=np.float32)
    wcat[:F, 0:H] = bases1
    wcat[:F, H:128] = root1
    wcat[F, H:128] = bias1
    wc_dev = np.ascontiguousarray(
        wcat.reshape(KCH, 128, 128).transpose(1, 0, 2)).astype(np.float16)

    for c in range(NCORES):
        ltf = np.zeros(L, np.int64)
        w1f = np.zeros(L, np.float32)
        w2f = np.zeros(L, np.float32)
        gsf = np.zeros(L, np.int64)
        for b in range(NBLK):
            kidx = c * NBLK + b
            sl = order[starts[kidx]:starts[kidx + 1]]
            off = chunk0[b] * 128
            ltf[off:off + len(sl)] = lt[sl]
            w1f[off:off + len(sl)] = w1[sl]
            w2f[off:off + len(sl)] = w2[sl]
            gsf[off:off + len(sl)] = gsrc_pos[sl]

        nreal = min(NPC, N - c * NPC)
        xp = np.zeros((FP, PADN), dtype=np.float32)
        xp[:F, :nreal] = x[c * NPC:c * NPC + nreal].T
        xp[F, :nreal] = 1.0
        xt_dev = np.ascontiguousarray(
            xp.reshape(KCH, 128, NBLK, 128).transpose(1, 2, 0, 3)
        ).astype(np.float16)

        d0f = np.zeros(DPAD, np.int64)
        d1f = np.zeros(DPAD, np.int64)
        d0f[:DPC] = remap[dec[0, c * DPC:(c + 1) * DPC]]
        d1f[:DPC] = remap[dec[1, c * DPC:(c + 1) * DPC]]

        blobc = np.zeros((128, TOTC), np.float16)
        blobc[:, XT0:XT0 + NBLK * KCH * 128] = xt_dev.reshape(128, -1)
        blobc[:, WC0:WC0 + KCH * 128] = wc_dev.reshape(128, -1)
        blobc[:, LT0:LT0 + CH] = ltf.reshape(CH, 128).T
        blobc[:, W10:W10 + CH] = w1f.reshape(CH, 128).T
        blobc[:, W20:W20 + CH] = w2f.reshape(CH, 128).T
        blobc[:, GI0:GI0 + CH * 8] = _wrap_idx(gsf).view(np.float16)
        blobc[:, D00:D00 + DPAD // 16] = _wrap_idx(d0f).view(np.float16)
        blobc[:, D10:D10 + DPAD // 16] = _wrap_idx(d1f).view(np.float16)
        blobc[0:H, R20:R20 + O] = root2.astype(np.float16)
        blobc[H, R20:R20 + O] = bias2.astype(np.float16)
        blobc[0:H, B20:B20 + O] = bases2.astype(np.float16)
        blobc[0:1, WT0:WT0 + 32] = np.ascontiguousarray(
            bil_w.reshape(1, O * O)).view(np.float16)

        in_maps.append({"blob": blobc})
    return in_maps, CH, ch_counts


def _bench(nc, in_maps, iters=20):
    import time as _time
    import jax
    from jax.sharding import Mesh, PartitionSpec, NamedSharding
    from jax.experimental.shard_map import shard_map
    from concourse import bass2jax as b2j

    b2j.install_neuronx_cc_hook()
    pname = nc.partition_id_tensor.name if nc.partition_id_tensor else None
    in_names, out_names, out_avals, zero_outs = [], [], [], []
    for alloc in nc.m.functions[0].allocations:
        if not isinstance(alloc, mybir.MemoryLocationSet):
            continue
        name = alloc.memorylocations[0].name
        if alloc.kind == "ExternalInput":
            if name != pname:
                in_names.append(name)
        elif alloc.kind == "ExternalOutput":
            shape = tuple(alloc.tensor_shape)
            dtype = mybir.dt.np(alloc.dtype)
            out_names.append(name)
            out_avals.append(jax.core.ShapedArray(shape, dtype))
            zero_outs.append(np.zeros(shape, dtype))
    n_params = len(in_names)
    n_outs = len(out_avals)
    in_names.extend(out_names)
    if pname is not None:
        in_names.append(pname)

    def _body(*args):
        operands = list(args)
        if pname is not None:
            operands.append(b2j.partition_id_tensor())
        return tuple(b2j._bass_exec_p.bind(
            *operands, out_avals=tuple(out_avals), in_names=tuple(in_names),
            out_names=tuple(out_names), lowering_input_output_aliases=(),
            sim_require_finite=True, sim_require_nnan=True, nc=nc))

    devices = jax.devices()[:NCORES]
    mesh = Mesh(np.asarray(devices), ("core",))
    specs = (PartitionSpec("core"),)
    fn = jax.jit(shard_map(_body, mesh=mesh,
                           in_specs=specs * (n_params + n_outs),
                           out_specs=specs * n_outs, check_rep=False),
                 keep_unused=True)
    concat_in = [np.concatenate([np.asarray(in_maps[c][nm])
                                 for c in range(NCORES)], axis=0)
                 for nm in in_names[:n_params]]
    sh = NamedSharding(mesh, PartitionSpec("core"))
    dev_in = [jax.device_put(a, sh) for a in concat_in]
    dev_zero = [jax.device_put(
        np.zeros((NCORES * z.shape[0], *z.shape[1:]), z.dtype), sh)
        for z in zero_outs]
    jax.block_until_ready(dev_in)
    jax.block_until_ready(dev_zero)
    # Under the axon tunnel a blocking round trip costs ~40-85 ms of pure
    # network latency regardless of kernel content, so per-iteration
    # blocking wall time measures the tunnel, not the hardware. Amortize:
    # dispatch a deep pipeline of executions, divide total wall time.
    t0 = _time.perf_counter()
    outs = fn(*dev_in, *dev_zero)
    jax.block_until_ready(outs)
    times = [_time.perf_counter() - t0]
    npipe = max(1024, iters)
    for _ in range(5):
        t0 = _time.perf_counter()
        allouts = [fn(*dev_in, *dev_zero) for _ in range(npipe)]
        jax.block_until_ready(allouts)
        times.append((_time.perf_counter() - t0) / npipe)
        del allouts
    del outs
    return times


def kernel(**inputs):
    in_maps, CH, ch_counts = _prep(inputs)
    nc = _build(CH, ch_counts)
    res = run_bass_kernel_spmd(nc, in_maps, core_ids=list(range(NCORES)),
                               trace=TRACE)
    globals()["LAST_RESULT"] = res
    if BENCH:
        times = _bench(nc, in_maps, iters=BENCH)
        globals()["LAST_TIMES"] = times

    out = np.empty(DEC, dtype=np.float32)
    for c in range(NCORES):
        arr = res.results[c]["sig"]  # [128, DCH]; slot s=ch*128+p -> edge s
        out[c * DPC:(c + 1) * DPC] = arr.T.reshape(-1)[:DPC]
    return out
